# revision 2
# baseline (speedup 1.0000x reference)
"""Trainium2 Bass kernel: 3-layer LSTM (input=1, hidden=32) + FC head.

Measured-cost-driven design:
  - Matmuls: 32x32 diagonal PE tiles (concurrent across tile positions,
    ~free on HW) — v1's scheme, in fp16.
  - ACT: per-gate sigmoid/tanh with per-partition bias APs ([128,512] each,
    489ns measured; merging gains nothing).
  - KEY FIX vs v1: the ACT engine is the bottleneck (~1.9ms floor) and its
    queue is FIFO. v1 emitted tanh_c right after the gate sigmoids of the
    same group; tanh_c waits on 3 DVE ops, head-of-line-blocking the next
    group's sigmoids. v3 software-pipelines: tanh_c + h-mult of group k are
    emitted after the sigmoids of group k+1, so ACT never stalls.
  - fp16 everywhere (h, c, S, x, weights): DVE 2x mode + better accuracy
    than bf16.

Sharding: pure data parallel, 8192 seqs/core; 4 streams x 4 subtiles x 512.
"""

import numpy as np

B, C, HS, WS = 32, 2, 32, 32
T = 64
H = 32
NCORES = 8
NSEQ = B * C * HS * WS          # 65536
NPC = NSEQ // NCORES            # 8192
NSTREAM = 4
NSUB = 4
FD = 512
TC = 8
REPS = 1                        # on-device repetitions (timing only)
PARTS = "full"                  # "mm" | "mm_act" | "full"
LAG = 2                         # stage_b software-pipeline depth (groups)
HMUL_GP = True                  # h = so*tanh(c) on GPSIMD instead of DVE
TMP_GP = False                  # GPSIMD too slow for the c-critical chain

_CACHE = {}


def _build_bass():
    NCH = T // TC
    import sys
    if '/opt/trn_rl_repo' not in sys.path:
        sys.path.insert(0, '/opt/trn_rl_repo')
    import concourse.bacc as bacc
    import concourse.mybir as mybir
    from concourse.tile import TileContext

    F32 = mybir.dt.float32
    FP16 = mybir.dt.float16
    AF = mybir.ActivationFunctionType
    OP = mybir.AluOpType

    nc = bacc.Bacc("TRN2", target_bir_lowering=False, debug=False)

    xin = nc.declare_dram_parameter("xin", [NCH, NSUB, NSTREAM, TC, FD], FP16,
                                    isOutput=False)
    wts = nc.declare_dram_parameter("wts", [128, 9 * 128], FP16, isOutput=False)
    bia = nc.declare_dram_parameter("bia", [128, 12], F32, isOutput=False)
    fcw = nc.declare_dram_parameter("fcw", [128, 1], FP16, isOutput=False)
    fcb = nc.declare_dram_parameter("fcb", [128, 1], F32, isOutput=False)
    y = nc.declare_dram_parameter("y", [NSTREAM, NSUB, FD], F32, isOutput=True)

    with TileContext(nc) as tc:
        with (
            tc.sbuf_pool(name="per", bufs=1) as per,
            tc.sbuf_pool(name="trans", bufs=6) as trans,
            tc.psum_pool(name="ps", bufs=8) as ps,
        ):
            wts_sb = per.tile([128, 9 * 128], FP16)
            bia_sb = per.tile([128, 12], F32)
            fcw_sb = per.tile([128, 1], FP16)
            fcb_sb = per.tile([128, 1], F32)
            nc.sync.dma_start(out=wts_sb[:], in_=wts[:])
            nc.sync.dma_start(out=bia_sb[:], in_=bia[:])
            nc.sync.dma_start(out=fcw_sb[:], in_=fcw[:])
            nc.sync.dma_start(out=fcb_sb[:], in_=fcb[:])

            h_t = [[per.tile([128, FD], FP16, name=f"h_{l}_{s}", tag=f"h_{l}_{s}")
                    for s in range(NSTREAM)] for l in range(3)]
            c_t = [[per.tile([128, FD], FP16, name=f"c_{l}_{s}", tag=f"c_{l}_{s}")
                    for s in range(NSTREAM)] for l in range(3)]
            xt = [per.tile([128, TC * FD], FP16, name=f"xt{i}", tag=f"xt{i}")
                  for i in range(2)]
            y_sb = per.tile([128, NSTREAM * FD], F32)

            if PARTS != "full":
                for l in range(3):
                    for s in range(NSTREAM):
                        nc.vector.memset(h_t[l][s][:], 0.0)
                        nc.vector.memset(c_t[l][s][:], 0.0)

            def load_chunk(k):
                for j in range(NSUB):
                    nc.sync.dma_start(
                        out=xt[k % 2][32 * j:32 * j + NSTREAM, :],
                        in_=xin[k, j].rearrange("s tc n -> s (tc n)"),
                    )

            load_chunk(0)

            def stage_a(l, s, t):
                """matmuls + gate activations + c-update; returns so tile."""
                gates = [ps.tile([128, FD], F32, name=f"g{l}_{s}_{t}_{g}",
                                 tag="gate") for g in range(4)]
                hl = h_t[l][s]
                for which in (0, 1):
                    for g in range(4):
                        for j in range(4):
                            pj = slice(32 * j, 32 * j + 32)
                            tp = (32 * j, 32 * j)
                            if which == 0:
                                if t > 0:
                                    nc.tensor.matmul(
                                        gates[g][pj, :],
                                        wts_sb[pj, l * 128 + 32 * g:
                                               l * 128 + 32 * g + 32],
                                        hl[pj, :],
                                        start=True, stop=False, tile_position=tp,
                                    )
                            elif l == 0:
                                k = t // TC
                                off = (t % TC) * FD
                                nc.tensor.matmul(
                                    gates[g][pj, :],
                                    wts_sb[32 * j:32 * j + NSTREAM,
                                           (5 + s) * 128 + 32 * g:
                                           (5 + s) * 128 + 32 * g + 32],
                                    xt[k % 2][32 * j:32 * j + NSTREAM,
                                              off:off + FD],
                                    start=(t == 0), stop=True, tile_position=tp,
                                )
                            else:
                                nc.tensor.matmul(
                                    gates[g][pj, :],
                                    wts_sb[pj, (2 + l) * 128 + 32 * g:
                                           (2 + l) * 128 + 32 * g + 32],
                                    h_t[l - 1][s][pj, :],
                                    start=(t == 0), stop=True, tile_position=tp,
                                )

                if PARTS == "mm":
                    return None

                def bap(g):
                    return bia_sb[:, l * 4 + g: l * 4 + g + 1]

                sig_i = trans.tile([128, FD], FP16, name=f"si{l}_{s}_{t}", tag="si")
                tan_g = trans.tile([128, FD], FP16, name=f"tg{l}_{s}_{t}", tag="tg")
                sig_o = trans.tile([128, FD], FP16, name=f"so{l}_{s}_{t}", tag="so")
                nc.scalar.activation(sig_i[:], gates[0][:], AF.Sigmoid, bias=bap(0))
                nc.scalar.activation(tan_g[:], gates[2][:], AF.Tanh, bias=bap(2))
                if t > 0:
                    sig_f = trans.tile([128, FD], FP16, name=f"sf{l}_{s}_{t}",
                                       tag="sf")
                    nc.scalar.activation(sig_f[:], gates[1][:], AF.Sigmoid,
                                         bias=bap(1))
                nc.scalar.activation(sig_o[:], gates[3][:], AF.Sigmoid, bias=bap(3))

                if PARTS == "mm_act":
                    return sig_o

                ct = c_t[l][s]
                if t == 0:
                    nc.vector.tensor_tensor(ct[:], sig_i[:], tan_g[:], OP.mult)
                else:
                    tmp = trans.tile([128, FD], FP16, name=f"tm{l}_{s}_{t}",
                                     tag="tm")
                    if TMP_GP:
                        nc.gpsimd.tensor_tensor(tmp[:], sig_i[:], tan_g[:],
                                                OP.mult)
                    else:
                        nc.vector.tensor_tensor(tmp[:], sig_i[:], tan_g[:],
                                                OP.mult)
                    nc.vector.tensor_tensor(ct[:], sig_f[:], ct[:], OP.mult)
                    nc.vector.tensor_tensor(ct[:], ct[:], tmp[:], OP.add)
                return sig_o

            def stage_b(pb):
                """tanh(c) + h = sig_o * tanh(c); lagged one group behind."""
                l, s, t, so = pb
                tan_c = trans.tile([128, FD], FP16, name=f"tc{l}_{s}_{t}", tag="tc")
                nc.scalar.activation(tan_c[:], c_t[l][s][:], AF.Tanh)
                if HMUL_GP:
                    nc.gpsimd.tensor_tensor(h_t[l][s][:], so[:], tan_c[:], OP.mult)
                else:
                    nc.vector.tensor_tensor(h_t[l][s][:], so[:], tan_c[:], OP.mult)

            def fc_head(s):
                pfc = ps.tile([128, FD], F32, name=f"pfc{s}", tag="gate")
                for j in range(4):
                    pj = slice(32 * j, 32 * j + 32)
                    nc.tensor.matmul(
                        pfc[32 * j:32 * j + 1, :], fcw_sb[pj, 0:1],
                        h_t[2][s][pj, :],
                        start=True, stop=True, tile_position=(32 * j, 32 * j),
                    )
                for j in range(4):
                    r = slice(32 * j, 32 * j + 1)
                    nc.scalar.activation(
                        y_sb[r, s * FD:(s + 1) * FD], pfc[r, :], AF.Identity,
                        bias=fcb_sb[r, :],
                    )
                for j in range(4):
                    nc.sync.dma_start(
                        out=y[s, j:j + 1, :],
                        in_=y_sb[32 * j:32 * j + 1, s * FD:(s + 1) * FD],
                    )

            for _rep in range(REPS):
                pending = []
                for tau in range(T + 2):
                    if tau % TC == TC // 2 and tau // TC + 1 < NCH:
                        load_chunk(tau // TC + 1)
                    for l in (2, 1, 0):
                        t = tau - l
                        if not (0 <= t < T):
                            continue
                        for s in range(NSTREAM):
                            so = stage_a(l, s, t)
                            if PARTS == "full":
                                pending.append((l, s, t, so))
                                if len(pending) > LAG:
                                    stage_b(pending.pop(0))
                if PARTS == "full":
                    for pb in pending:
                        stage_b(pb)
                    for s in range(NSTREAM):
                        fc_head(s)

    nc.compile()
    return nc


def _prep_inputs(x, w_ih0, w_hh0, b_ih0, b_hh0, w_ih1, w_hh1, b_ih1, b_hh1,
                 w_ih2, w_hh2, b_ih2, b_hh2, fc_w, fc_b):
    FP16 = np.float16
    NCH = T // TC
    x_flat = np.ascontiguousarray(x, dtype=np.float32).reshape(NSEQ, T)
    w_hh = [w_hh0, w_hh1, w_hh2]
    w_ih = [w_ih0, w_ih1, w_ih2]
    b_sum = [b_ih0 + b_hh0, b_ih1 + b_hh1, b_ih2 + b_hh2]

    wts = np.zeros((9, 128, 128), np.float32)
    for l in range(3):
        blk = np.asarray(w_hh[l], np.float32).reshape(128, 32).T  # [k, 4H]
        for j in range(4):
            wts[l, 32 * j:32 * j + 32, :] = blk
    for l in (1, 2):
        blk = np.asarray(w_ih[l], np.float32).reshape(128, 32).T
        for j in range(4):
            wts[2 + l, 32 * j:32 * j + 32, :] = blk
    w0 = np.asarray(w_ih0, np.float32).reshape(128)
    for s in range(4):
        for j in range(4):
            wts[5 + s, 32 * j + s, :] = w0

    bia = np.zeros((128, 12), np.float32)
    for l in range(3):
        bb = np.asarray(b_sum[l], np.float32).reshape(4, 32)
        for g in range(4):
            for j in range(4):
                bia[32 * j:32 * j + 32, l * 4 + g] = bb[g]

    fcw = np.zeros((128, 1), np.float32)
    fw = np.asarray(fc_w, np.float32).reshape(32)
    for j in range(4):
        fcw[32 * j:32 * j + 32, 0] = fw
    fcb = np.full((128, 1), np.float32(np.asarray(fc_b).reshape(())), np.float32)
    wts_packed = np.ascontiguousarray(
        wts.transpose(1, 0, 2).reshape(128, 9 * 128)).astype(FP16)
    fcw16 = fcw.astype(FP16)

    in_maps = []
    for core in range(NCORES):
        xc = x_flat[core * NPC:(core + 1) * NPC]
        xv = xc.reshape(NSTREAM, NSUB, FD, NCH, TC)
        xk = np.ascontiguousarray(xv.transpose(3, 1, 0, 4, 2)).astype(FP16)
        in_maps.append({"xin": xk, "wts": wts_packed, "bia": bia,
                        "fcw": fcw16, "fcb": fcb})
    return in_maps


def _run(in_maps, trace=False):
    import sys
    if '/opt/trn_rl_repo' not in sys.path:
        sys.path.insert(0, '/opt/trn_rl_repo')
    from concourse.bass_utils import run_bass_kernel_spmd
    if "nc" not in _CACHE:
        _CACHE["nc"] = _build_bass()
    nc = _CACHE["nc"]
    return run_bass_kernel_spmd(nc, in_maps, list(range(NCORES)), trace=trace)


def kernel(**inputs):
    in_maps = _prep_inputs(**inputs)
    res = _run(in_maps)
    outs = []
    for core in range(NCORES):
        yc = res.results[core]["y"]
        outs.append(yc.reshape(NPC))
    return np.concatenate(outs).reshape(B, C, HS, WS).astype(np.float32)
